# revision 1
# baseline (speedup 1.0000x reference)
"""Trainium2 Bass kernel for nn_Attention_43190191129190.

Model (per batch element b of 8):
    y   = x + dwconv3x3(x) + conv_b          (depthwise residual positional conv)
    qkv = y @ qkv_w.T ; split into q, k, v   (8 heads, dim 32)
    out = softmax(q k^T / sqrt(32)) v
    out = out @ out_w.T + out_b

Sharding: pure data-parallel, one batch element per NeuronCore (8 cores).

Per-core design (v4 — thin-output PV, half-width S/exp pipeline):

  1. x arrives bf16, spatially pre-transposed on the host; two DMA-xbar
     transpose calls land it directly in a zero-haloed [C, 34, 34] x^T
     image in SBUF (no PE transposes, no staging tiles).
  2. conv: per (128-channel tile, 512-token half), 9 bf16 matmuls with
     diagonal weight matrices accumulated in a 1-bank PSUM half; the
     PSUM->SBUF evacuation adds the conv bias (per-partition
     tensor_scalar). -> y^T f32r.
  3. q^T/k^T [feature, token] f32r via qkv_w^T chunks against y^T;
     v in [token, feature] bf16 with a ones column interleaved per head
     ([v_h|1] 33-wide tiles per 128-token chunk).
  4. Attention, head pair per generation, 8 token-chunk (m) steps each:
       S^T[m,n] = k_h^T.T @ q_h^T (f32r), one 512-wide PSUM HALF-tile per
       (head, j): 4 half tiles per m-step rotating through 6 one-bank
       slots, so a slot's reuse never waits on the *previous* step's exp.
       exp per half on a per-(head,m,j) engine:
         ACT: activation Exp -> bf16;
         DVE/Pool: Schraudolph fast-exp — one tensor_scalar
         (s*A + B) -> int16 whose bits ARE bf16(exp(s*SCALE)); ~1-2%
         error on those slots, mostly cancelled by the softmax
         denominator (the ones column sums the same approximated p).
       PV (thin output): out[n,d] per head accumulates in a per-head
       [128, 8x33] PSUM bank — stationary = p^T 128-token chunk, moving =
       [v_h|1] (33 wide, bf16, ~14ns/matmul); column 32 accumulates the
       softmax denominators. One accumulation group per bank (start only
       on the first write — the lazy 2KB region-zeroing covers the other
       sub-regions).
       Normalization is per-PARTITION: one [128,8] reciprocal and one
       fused tensor_tensor multiply (denominator broadcast via a 0-stride
       free dim) per head -> a_sb [n, head*32+d] bf16.
  5. a_sb token-chunks are PE-transposed back (bf16) to attnT [(h d), n]
     and projected: chunk-1 (heads 4-7) + out_b staged mid-kernel into
     partial1; chunk-0 + final add + store in the tail.

Pre-attention work (qkv halves, v chunks, chunk-1 projections, a_sb
transposes) is interleaved one self-contained slice per m-step into the
pair loops (alloc+use+evacuate within the slice — holding a PSUM tile
across steps can head-of-line-deadlock the in-order PE queue).
PSUM budget: 6x[128,512]f32 half slots + 2x[128,264]f32 PV accumulators.
"""

import os

import numpy as np

import concourse.bass as bass
import concourse.tile as tile
from concourse import bacc, mybir
from concourse.bass_utils import run_bass_kernel_spmd

F32 = mybir.dt.float32
F32R = mybir.dt.float32r
BF16 = mybir.dt.bfloat16
I16 = mybir.dt.int16
AF = mybir.ActivationFunctionType
ALU = mybir.AluOpType

B, N, C = 8, 1024, 256
HEADS, DH = 8, 32
SCALE = DH ** -0.5
PAD = 34  # 32x32 spatial grid with 1-px halo
# packed constant blob column offsets (bf16 elements)
# row0 layout: outb [0:256] | ones [256:768] | convb row [768:1024]
BW18, BID, BOWT, BROW0 = 0, 18, 146, 662
BLOBW = 662 + 1024

TAPS = [(ky, kx) for ky in range(3) for kx in range(3)]
# chunk-1 head pairs first so the chunk-1 projection can run mid-kernel;
# the tail then only waits on the last pair's (chunk-0) normalization
PAIRS = [(5, 7), (4, 6), (1, 3), (0, 2)]

# Schraudolph fast-exp: int16 bits of bf16(exp(s*SCALE)) = s*A + B
SCHR_C = 450000.0
SCHR_A = float(SCALE * (2 ** 23) / np.log(2) / 65536.0)
SCHR_B = float((127 * 2 ** 23 - SCHR_C) / 65536.0)

# exp engine per (head slot, m, j): A=ACT exact, V=DVE (Schraudolph)
EXPH = {
    (0, "j0"): ["A"] * 8,
    (0, "j1"): ["A", "V", "A", "V", "A", "V", "A", "V"],
    (1, "j0"): ["V"] * 8,
    (1, "j1"): ["V", "A", "V", "A", "V", "A", "V", "A"],
}


def exp_engine(hslot, m, j):
    return EXPH[(hslot, f"j{j}")][m]


def build_nc(debug_dump=False):
    nc = bacc.Bacc("TRN2", target_bir_lowering=False, debug=False, num_devices=8)

    x_d = nc.dram_tensor("x", (N, C), BF16, kind="ExternalInput").ap()
    qkvwT_d = nc.dram_tensor("qkv_wT", (C, 3 * C), F32R, kind="ExternalInput").ap()
    # all small constants packed in one [128, BLOB] bf16 DMA:
    # w18 [128,18] | id [128,128] | outwT [128,512] | convb(f32 bits) [128,4]
    # | row0: outb [1,256] + ones [1,128]
    blob_d = nc.dram_tensor("blob", (128, BLOBW), BF16, kind="ExternalInput").ap()
    out_d = nc.dram_tensor("out", (N, C), F32, kind="ExternalOutput").ap()
    dbg = {}
    if debug_dump:
        for name, shape in (
            ("d_yT", (128, 2, N)), ("d_qT", (128, 2, N)), ("d_kT", (128, 2, N)),
            ("d_v", (128, 8, 264)), ("d_asb", (128, 8, 256)),
        ):
            dbg[name] = nc.dram_tensor(name, shape, F32, kind="ExternalOutput").ap()

    with tile.TileContext(nc) as tc:
        with (
            tc.tile_pool(name="const", bufs=1) as const,
            tc.tile_pool(name="big", bufs=1) as big,
            tc.tile_pool(name="pT", bufs=20) as ppool,
            tc.tile_pool(name="rcp", bufs=4) as rcp_p,
            tc.tile_pool(name="outs", bufs=3) as outs_p,
            tc.tile_pool(name="pst", bufs=6, space="PSUM") as pst,
            tc.tile_pool(name="pap", bufs=2, space="PSUM") as pap,
        ):
            # ---- persistent activations (x image first: DMA critical path)
            xpadT = big.tile([128, 2, PAD * PAD], BF16, tag="xpadT")
            xpv = xpadT.bitcast(mybir.dt.uint16).rearrange(
                "p ct (h w) -> p ct h w", h=PAD
            )
            nc.vector.memset(xpv[:, :, 0, :], 0)
            nc.vector.memset(xpv[:, :, PAD - 1, :], 0)
            nc.vector.memset(xpv[:, :, :, 0], 0)
            nc.vector.memset(xpv[:, :, :, PAD - 1], 0)

            # ---- warm-ups first: the exp ACT-table load and a few tiny
            # PE matmuls (starts the pstate ramp clock before the real
            # matmuls arrive) happen while the DMAs stream in
            zerob_sb = const.tile([128, 1], F32, tag="zerob")
            nc.vector.memset(zerob_sb, 0.0)
            warm_sb = const.tile([1, 1], F32, tag="warm")
            nc.scalar.activation(
                warm_sb, zerob_sb[0:1, 0:1], AF.Exp,
                bias=zerob_sb[0:1], scale=1.0,
            )
            # PE warm-up bridge: a chained trickle of 1-wide matmuls keeps
            # the PE "recently active" through the DMA wait so the conv burst
            # is not dispatched into the cost model's cold p-state
            wv = const.tile([1, 20], F32, tag="wv")
            nc.vector.memset(wv, 0.0)
            for k in range(17):
                wps = pst.tile([128, 256], F32, tag="ps", name="wps")
                nc.tensor.matmul(
                    wps[0:1, 0:1], lhsT=wv[0:1, k:k + 1],
                    rhs=wv[0:1, k:k + 1], start=True, stop=True,
                )
                if k + 1 < 20:
                    nc.vector.tensor_copy(wv[0:1, k + 1:k + 2], wps[0:1, 0:1])

            # ---- DMAs: x + conv inputs first, halves split across the two
            # hardware queues so neither serializes the conv start. The
            # diagonal conv matrices are generated ON DEVICE (affine_select
            # from a tiny [128,18] tap table) instead of DMAing 294KB.
            blob_sb = const.tile([128, BLOBW], BF16, tag="blob")
            nc.sync.dma_start(blob_sb, blob_d)
            w18_sb = blob_sb[:, BW18:BW18 + 18]
            id_sb = blob_sb[:, BID:BID + 128]
            outwT_sb = blob_sb[:, BOWT:BOWT + 512].rearrange(
                "p (kc f) -> p kc f", kc=2)
            outb_sb = blob_sb[0:1, BROW0:BROW0 + 256]
            ones_sb = blob_sb[0:1, BROW0 + 256:BROW0 + 768]
            convbr_sb = blob_sb[0:1, BROW0 + 768:BROW0 + 1024]
            diag_sb = const.tile([128, 18, 128], BF16, tag="diag")
            # one [128,128] diagonal per tap (pipelines ahead of the conv
            # matmuls; a single [128,18,128] affine_select would gate the
            # first tap on all 2304 columns)
            for idx in range(18):
                nc.gpsimd.affine_select(
                    diag_sb[:, idx, :],
                    bass.AP(tensor=w18_sb.tensor,
                            offset=w18_sb.offset + idx,
                            ap=[list(w18_sb.ap[0]), [0, 128]]),
                    pattern=[[1, 128]],
                    compare_op=ALU.is_equal,
                    fill=0.0,
                    base=0,
                    channel_multiplier=-1,
                )
            qkvwT_sb = const.tile([128, 2, 3 * C], F32R, tag="qkvwT")
            # the xbar DMA transpose is only bit-exact on hardware for a
            # contiguous 2D destination: stage x^T, then copy the 32x32
            # interior into the zero-haloed image on the (startup-idle) ACT
            # all input DMAs ride the sync queue in priority order: a DMA
            # parked on the scalar queue blocks the ACT engine's SEQ, and
            # the DMA_ENGINES transfer slot is serial anyway
            xstg = big.tile([128, 2, N], BF16, tag="xstg")
            for ct in range(2):
                nc.sync.dma_start_transpose(
                    xstg[:, ct, :], x_d[:, ct * 128:(ct + 1) * 128],
                )
            for ct in range(2):
                nc.sync.dma_start(
                    qkvwT_sb[:, ct, :],
                    qkvwT_d[ct * 128:(ct + 1) * 128, :],
                )
            for ct in range(2):
                nc.vector.tensor_copy(
                    xpadT[:, ct, :].rearrange("p (h w) -> p h w", h=PAD)[
                        :, 1:33, 1:33
                    ],
                    xstg[:, ct, :].rearrange("p (h w) -> p h w", h=32),
                )

            # f32r identity: lets the tail fold `+ partial1` into the
            # projection PSUM group as a K=128 matmul instead of a DVE add
            idr_sb = const.tile([128, 128], F32R, tag="idr")
            nc.scalar.copy(idr_sb, id_sb)
            yT = big.tile([128, 2, N], F32R, tag="yT")
            qT = big.tile([128, 2, N], F32R, tag="qT")
            kT = big.tile([128, 2, N], F32R, tag="kT")
            # [v_h | 1] per (token-chunk, head); ones preset via memset
            vsb = big.tile([128, 8, 8 * 33], BF16, tag="v")
            nc.gpsimd.memset(vsb, 1.0)
            a_sb = big.tile([128, 8, 256], BF16, tag="a_sb")
            attnT = big.tile([128, 2, N], BF16, tag="attnT")
            partial1 = big.tile([128, 8, C], F32R, tag="partial1")

            # psum evacuations: GPSIMD cannot access PSUM on HW, so they
            # alternate between the ACT (scalar.copy) and DVE engines
            _cp = [0]

            def copy_alt(dst, src_ap):
                _cp[0] += 1
                if _cp[0] % 2:
                    nc.scalar.copy(dst, src_ap)
                else:
                    nc.vector.tensor_copy(dst, src_ap)

            # ---- conv: 9 diagonal matmuls per (channel tile, token half)
            def emit_conv_half(ct, j):
                cps = pst.tile([128, 512], F32, tag="ps", name=f"cacc{ct}{j}")
                view = xpadT[:, ct, :].rearrange("p (h w) -> p h w", h=PAD)
                for t, (ky, kx) in enumerate(TAPS):
                    nc.tensor.matmul(
                        cps,
                        lhsT=diag_sb[:, ct * 9 + t, :],
                        rhs=view[:, ky + 16 * j: ky + 16 * j + 16, kx: kx + 32],
                        start=(t == 0),
                        stop=False,
                    )
                # conv bias as a 10th K=1 tap (GPSIMD cannot touch PSUM)
                nc.tensor.matmul(
                    cps,
                    lhsT=convbr_sb[0:1, ct * 128:(ct + 1) * 128],
                    rhs=ones_sb,
                    start=False,
                    stop=True,
                )
                nc.scalar.copy(yT[:, ct, j * 512:(j + 1) * 512], cps)

            for ct in range(2):
                for j in range(2):
                    emit_conv_half(ct, j)

            # ---- q^T / k^T feature-tile halves + v chunks ----
            def emit_qk_half(ft, j):
                dstT, dc = (qT, ft) if ft < 2 else (kT, ft - 2)
                fofs = 0 if ft < 2 else 256
                qps = pst.tile([128, 512], F32, tag="ps", name="qps")
                for kc in range(2):
                    nc.tensor.matmul(
                        qps,
                        lhsT=qkvwT_sb[:, kc, fofs + dc * 128: fofs + (dc + 1) * 128],
                        rhs=yT[:, kc, j * 512:(j + 1) * 512],
                        start=(kc == 0),
                        stop=(kc == 1),
                    )
                copy_alt(dstT[:, dc, j * 512:(j + 1) * 512], qps)

            def emit_v(nt):
                vps = pst.tile([128, 256], F32, tag="ps", name="vps")
                for kc in range(2):
                    nc.tensor.matmul(
                        vps,
                        lhsT=yT[:, kc, nt * 128:(nt + 1) * 128],
                        rhs=qkvwT_sb[:, kc, 512:768],
                        start=(kc == 0),
                        stop=(kc == 1),
                    )
                vv = vsb[:, nt, :].rearrange("p (hh c) -> p hh c", c=33)
                sv = vps.rearrange("p (hh c) -> p hh c", c=32)
                copy_alt(vv[:, :, 0:32], sv)

            # pair 0 needs q/k feature chunk 1 (heads 4-7); all v chunks are
            # evacuated up front so the pair-0 m-steps stay copy-free on the
            # exp engines
            for j in range(2):
                emit_qk_half(1, j)
            for j in range(2):
                emit_qk_half(3, j)
            for nt in range(8):
                emit_v(nt)

            # ---- a_sb -> attnT transposes (post-normalization) ----
            def emit_atr(ct, nc_i):
                tp = pst.tile([128, 256], BF16, tag="ps", name="atp")
                nc.tensor.transpose(
                    tp[:, 0:128],
                    a_sb[:, nc_i, ct * 128:(ct + 1) * 128],
                    id_sb,
                )
                copy_alt(attnT[:, ct, nc_i * 128:(nc_i + 1) * 128], tp[:, 0:128])

            def emit_proj1(nt):
                pj = pst.tile([128, 256], F32, tag="ps", name="pj1")
                nc.tensor.matmul(
                    pj,
                    lhsT=attnT[:, 1, nt * 128:(nt + 1) * 128],
                    rhs=outwT_sb[:, 1, :],
                    start=True,
                    stop=False,
                )
                nc.tensor.matmul(
                    pj,
                    lhsT=ones_sb[0:1, 0:128],
                    rhs=outb_sb,
                    start=False,
                    stop=True,
                )
                copy_alt(partial1[:, nt, :], pj)

            # interleaved extras, one self-contained slice per m-step
            def pair_extra(ip, m):
                if ip == 1:
                    if m < 2:
                        emit_qk_half(0, m)
                    elif m < 4:
                        emit_qk_half(2, m - 2)
                elif ip == 2:
                    if m >= 3:
                        emit_atr(1, m - 3)  # heads 4-7 ready after both of
                        # pair 1's deferred norms (carry slots m2 and m3)
                elif ip == 3:
                    if m < 3:
                        emit_atr(1, m + 5)
                    else:
                        emit_proj1(m - 3)

            # ---- exp half emission ----
            def emit_exp(eng, st):
                if eng == "A":
                    p = ppool.tile([128, 512], BF16, tag="pT", name="pA")
                    nc.scalar.activation(p, st, AF.Exp, bias=zerob_sb, scale=SCALE)
                    return p
                p = ppool.tile([128, 512], I16, tag="pT", name="pV")
                nc.vector.tensor_scalar(
                    out=p, in0=st, scalar1=SCHR_A, scalar2=SCHR_B,
                    op0=ALU.mult, op1=ALU.add,
                )
                return p.bitcast(BF16)

            # ---- attention ----
            def emit_pv(m, ph, pas, heads):
                # one accumulation group per pa bank: start only on the first
                # write (lazy 2KB region-zeroing covers the other 7
                # sub-regions), stop only on the last
                for nc_i in range(8):
                    j = nc_i // 4
                    for hs in (0, 1):
                        nc.tensor.matmul(
                            pas[hs][:, nc_i * 33: nc_i * 33 + 33],
                            lhsT=ph[(hs, j)][:, (nc_i % 4) * 128:(nc_i % 4 + 1) * 128],
                            rhs=vsb[:, m, 33 * heads[hs]: 33 * heads[hs] + 33],
                            start=(m == 0 and nc_i == 0),
                            stop=(m == 7 and nc_i == 7),
                        )

            def emit_norm(pas, heads):
                for h, pa in zip(heads, pas):
                    pav = pa.rearrange("p (nc e) -> p nc e", e=33)
                    rcp = rcp_p.tile([128, 8], F32, tag="rcp", name="rcp")
                    nc.vector.reciprocal(rcp, pav[:, :, 32])
                    rcp_b = bass.AP(
                        tensor=rcp.tensor, offset=rcp.offset,
                        ap=[list(rcp.ap[0]), [1, 8], [0, 32]],
                    )
                    nc.vector.tensor_tensor(
                        out=a_sb[:, :, h * 32: h * 32 + 32],
                        in0=pav[:, :, 0:32],
                        in1=rcp_b,
                        op=ALU.mult,
                    )

            carry = []
            for ip, (hA, hB) in enumerate(PAIRS):
                pas = (
                    pap.tile([128, 264], F32, tag="pa", name=f"paA{ip}"),
                    pap.tile([128, 264], F32, tag="pa", name=f"paB{ip}"),
                )
                heads = (hA, hB)
                pend = []
                for m in range(8):
                    ph = {}
                    for hs, h in ((0, hA), (1, hB)):
                        a = 32 * (h % 4)
                        hc = h // 4
                        for j in range(2):
                            st = pst.tile([128, 512], F32, tag="ps", name="st")
                            nc.tensor.matmul(
                                st,
                                lhsT=kT[a:a + 32, hc, m * 128:(m + 1) * 128],
                                rhs=qT[a:a + 32, hc, j * 512:(j + 1) * 512],
                                start=True,
                                stop=True,
                                tile_position=(a, 0),
                            )
                            ph[(hs, j)] = emit_exp(exp_engine(hs, m, j), st)
                    if carry:
                        carry.pop(0)()
                    pair_extra(ip, m)
                    pend.append((m, ph))
                    if len(pend) > 2:
                        emit_pv(*pend.pop(0), pas, heads)
                # defer the tail PVs + normalization into the next pair's
                # m-loop so the PE never waits on the trailing exps
                thunks = [
                    (lambda e=e, pas=pas, heads=heads: emit_pv(*e, pas, heads))
                    for e in pend
                ]
                for hs in (0, 1):
                    thunks.append(
                        lambda hs=hs, pas=pas, heads=heads: emit_norm(
                            (pas[hs],), (heads[hs],))
                    )
                carry = thunks

            # ---- tail: last pair's PVs + norm, remaining projections ----
            emit_proj1(5)
            carry.pop(0)()  # PV(6) of last pair
            emit_proj1(6)
            carry.pop(0)()  # PV(7)
            emit_proj1(7)
            for t in carry:  # the two norms
                t()

            if debug_dump:
                nc.sync.dma_start(dbg["d_yT"], yT.bitcast(F32))
                nc.sync.dma_start(dbg["d_qT"], qT.bitcast(F32))
                nc.sync.dma_start(dbg["d_kT"], kT.bitcast(F32))
                dvf = big.tile([128, 8, 264], F32, tag="dvf")
                nc.vector.tensor_copy(dvf, vsb)
                nc.sync.dma_start(dbg["d_v"], dvf)
                daf = big.tile([128, 8, 256], F32, tag="daf")
                nc.vector.tensor_copy(daf, a_sb)
                nc.sync.dma_start(dbg["d_asb"], daf)

            # transpose chunk-0, project, add staged half, store
            emit_atr(0, 0)
            emit_atr(0, 1)
            osb2 = None
            for nt in range(8):
                if nt + 2 < 8:
                    emit_atr(0, nt + 2)
                ops = pst.tile([128, 256], F32, tag="ps")
                nc.tensor.matmul(
                    ops,
                    lhsT=attnT[:, 0, nt * 128:(nt + 1) * 128],
                    rhs=outwT_sb[:, 0, :],
                    start=True,
                    stop=False,
                )
                # += partial1 via an identity matmul (frees the DVE tail)
                nc.tensor.matmul(
                    ops,
                    lhsT=idr_sb,
                    rhs=partial1[:, nt, :],
                    start=False,
                    stop=True,
                )
                if nt % 2 == 0:
                    osb2 = outs_p.tile([128, 2, C], F32, tag="o", name="osb2")
                copy_alt(osb2[:, nt % 2, :], ops)
                if nt % 2 == 1:
                    # one batched DMA per 2 token chunks (HWDGE overhead is
                    # per-descriptor-set, ~625ns each)
                    oq = nc.sync if (nt // 2) % 2 else nc.scalar
                    oq.dma_start(
                        out_d[(nt - 1) * 128:(nt + 1) * 128, :].rearrange(
                            "(c p) f -> p c f", p=128),
                        osb2,
                    )

    nc.compile()
    return nc


_NC = None
LAST_RESULTS = None


def _host_prep(conv_w, conv_b, qkv_w, out_w, out_b):
    import ml_dtypes

    conv_w = np.asarray(conv_w, np.float32).reshape(C, 3, 3)
    w18 = np.zeros((128, 18), np.float32)
    for ct in range(2):
        for t, (ky, kx) in enumerate(TAPS):
            d = conv_w[128 * ct: 128 * (ct + 1), ky, kx].copy()
            if (ky, kx) == (1, 1):
                d += 1.0  # residual connection folded into the center tap
            w18[:, ct * 9 + t] = d
    blob = np.zeros((128, BLOBW), ml_dtypes.bfloat16)
    blob[:, BW18:BW18 + 18] = w18.astype(ml_dtypes.bfloat16)
    blob[:, BID:BID + 128] = np.eye(128, dtype=ml_dtypes.bfloat16)
    owT = np.ascontiguousarray(np.asarray(out_w, np.float32).T).astype(
        ml_dtypes.bfloat16)  # [256 in, 256 outc]
    blob[:, BOWT:BOWT + 512] = np.concatenate(
        [owT[0:128, :], owT[128:256, :]], axis=1)
    blob[0, BROW0:BROW0 + 256] = np.asarray(out_b, np.float32).astype(
        ml_dtypes.bfloat16)
    blob[0, BROW0 + 256:BROW0 + 768] = np.ones(512, ml_dtypes.bfloat16)
    blob[0, BROW0 + 768:BROW0 + 1024] = np.asarray(
        conv_b, np.float32).astype(ml_dtypes.bfloat16)
    return {
        "qkv_wT": np.ascontiguousarray(np.asarray(qkv_w, np.float32).T),
        "blob": blob,
    }


def _prep_x(x):
    """bf16 copy for the conv path (the 2D xbar DMA transpose is exact)."""
    import ml_dtypes

    return np.ascontiguousarray(np.asarray(x, np.float32).astype(ml_dtypes.bfloat16))


def kernel(x, conv_w, conv_b, qkv_w, out_w, out_b):
    global _NC, LAST_RESULTS

    if _NC is None:
        _NC = build_nc()
    x = _prep_x(x)
    shared = _host_prep(conv_w, conv_b, qkv_w, out_w, out_b)
    in_maps = [{**shared, "x": np.ascontiguousarray(x[b])} for b in range(B)]
    trace = bool(int(os.environ.get("KERNEL_TRACE", "0")))
    try:
        res = run_bass_kernel_spmd(_NC, in_maps, core_ids=list(range(B)), trace=trace)
    except Exception:
        if not trace:
            raise
        res = run_bass_kernel_spmd(_NC, in_maps, core_ids=list(range(B)), trace=False)
    LAST_RESULTS = res
    return np.stack([res.results[b]["out"] for b in range(B)], axis=0)

